# revision 13
# baseline (speedup 1.0000x reference)
"""AutoRound GPTQ int4 linear on 8 TRN2 NeuronCores.

y = x @ dequant(qweight, qzeros, scales), column-parallel over out_features:
each core owns a [4096, 1376] weight shard, dequantizes it on-chip (int4
unpack + zero/scale in fp16), PE-transposes x tiles, and runs an fp16 matmul
with fp32 PSUM accumulation. x is replicated; outputs are concatenated.

Host-side marshaling (layout only, no arithmetic): qweight packed rows are
repeated 8x so row k holds the int32 containing weight row k; scales rows are
repeated 128x so row k holds the scale of k's quant group.
"""

import sys

sys.path.insert(0, "/opt/trn_rl_repo")

import numpy as np

import concourse.bacc as bacc
import concourse.mybir as mybir
import concourse.tile as tile
from concourse import masks
from concourse.bass_utils import run_bass_kernel_spmd

IN_F = 4096
OUT_F = 11008
G = 32  # quant groups (group size 128 == one k-tile)
N_CORES = 8
OUT_SHARD = OUT_F // N_CORES  # 1376
B, S = 4, 2048
M_ROWS = B * S  # 8192

f32 = mybir.dt.float32
f16 = mybir.dt.float16
i32 = mybir.dt.int32
Alu = mybir.AluOpType


def build_nc(m_rows=M_ROWS, out_shard=OUT_SHARD, in_f=IN_F):
    KT = in_f // 128  # k-tiles; each k-tile is exactly one quant group
    MT = m_rows // 128
    assert KT == G and KT % 4 == 0 and m_rows % 128 == 0 and out_shard % 8 == 0

    chunks = []
    o = 0
    while o < out_shard:
        w = min(512, out_shard - o)
        chunks.append((o, w))
        o += w

    nc = bacc.Bacc("TRN2", target_bir_lowering=False)
    x_d = nc.dram_tensor("x", (m_rows, in_f), f32, kind="ExternalInput")
    qw_d = nc.dram_tensor("qweight", (in_f, out_shard), i32, kind="ExternalInput")
    qz_d = nc.dram_tensor("qzeros", (G, out_shard // 8), i32, kind="ExternalInput")
    s_d = nc.dram_tensor("scales", (in_f, out_shard), f16, kind="ExternalInput")
    out_d = nc.dram_tensor("out", (m_rows, out_shard), f32, kind="ExternalOutput")

    with tile.TileContext(nc) as tc:
        with (
            tc.tile_pool(name="const", bufs=1) as cpool,
            tc.tile_pool(name="wpool", bufs=KT) as wpool,
            tc.tile_pool(name="qrep_p", bufs=2) as qrep_pool,
            tc.tile_pool(name="sb_p", bufs=2) as sb_pool,
            tc.tile_pool(name="row_p", bufs=2) as row_pool,
            tc.tile_pool(name="bcast_p", bufs=2) as bcast_pool,
            tc.tile_pool(name="xin_p", bufs=2) as xin_pool,
            tc.tile_pool(name="xt_p", bufs=2) as xt_pool,
            tc.tile_pool(name="out_p", bufs=2) as out_pool,
            tc.tile_pool(name="ptr", bufs=2, space="PSUM") as ptr_pool,
            tc.tile_pool(name="pout", bufs=3, space="PSUM") as pout_pool,
        ):
            # --- constants ---
            ident = cpool.tile([128, 128], f32, tag="ident")
            masks.make_identity(nc, ident[:])

            iota_t = cpool.tile([128, 1], i32, tag="iota")
            nc.gpsimd.iota(iota_t[:], pattern=[[0, 1]], base=0, channel_multiplier=4)
            # per-partition nibble shift: 4*(p % 8), int32 tensor operand
            shift_ap = cpool.tile([128, 1], i32, tag="shift")
            nc.vector.tensor_scalar(shift_ap[:], iota_t[:], 28, None, Alu.bitwise_and)

            qz_sb = cpool.tile([G, out_shard // 8], i32, tag="qz_sb")
            nc.sync.dma_start(qz_sb[:], qz_d[:])
            # unpack zeros along the free dim (int-only: bitvec ops cannot cast)
            z_sbi = cpool.tile([G, out_shard], i32, tag="z_sbi")
            z_r = z_sbi[:].rearrange("g (r i) -> g r i", i=8)
            for i in range(8):
                nc.vector.tensor_scalar(
                    z_r[:, :, i], qz_sb[:], 4 * i, 15,
                    Alu.logical_shift_right, Alu.bitwise_and,
                )
            z_sbh = cpool.tile([G, out_shard], f16, tag="z_sbh")
            nc.vector.tensor_copy(z_sbh[:], z_sbi[:])

            # --- dequantize weight shard into SBUF (fp16, [k, n] layout) ---
            w_tiles = []
            for t in range(KT):
                qrep = qrep_pool.tile([128, out_shard], i32, tag="qrep")
                nc.sync.dma_start(qrep[:], qw_d[128 * t : 128 * (t + 1), :])
                sb = sb_pool.tile([128, out_shard], f16, tag="sb")
                nc.sync.dma_start(sb[:], s_d[128 * t : 128 * (t + 1), :])
                zrow = row_pool.tile([1, out_shard], f16, tag="zrow")
                nc.sync.dma_start(zrow[:], z_sbh[t : t + 1, :])
                zb = bcast_pool.tile([128, out_shard], f16, tag="zb")
                nc.gpsimd.partition_broadcast(zb[:], zrow[:])
                # in-place int chain: q >>= shift; q &= 15
                nc.vector.tensor_tensor(
                    qrep[:], qrep[:],
                    shift_ap[:].broadcast_to((128, out_shard)),
                    Alu.logical_shift_right,
                )
                nc.vector.tensor_scalar(qrep[:], qrep[:], 15, None, Alu.bitwise_and)
                w_t = wpool.tile([128, out_shard], f16, tag="w")
                nc.scalar.copy(w_t[:], qrep[:])  # int32 -> fp16 (values 0..15)
                nc.vector.tensor_tensor(w_t[:], w_t[:], zb[:], Alu.subtract)
                nc.vector.tensor_tensor(w_t[:], w_t[:], sb[:], Alu.mult)
                w_tiles.append(w_t)

            # --- main loop over m-tiles ---
            # x is loaded as fp32 halves; PE transposes (fp32, interleaved with
            # the previous m-tile's matmuls to keep HAM warm) write PSUM, and
            # the ACT drain-copy casts to fp16 xT.
            def load_x(mi):
                halves = []
                for h in range(2):
                    xi = xin_pool.tile([128, in_f // 2], f32, tag="xin")
                    nc.sync.dma_start(
                        xi[:],
                        x_d[mi * 128 : (mi + 1) * 128,
                            h * (in_f // 2) : (h + 1) * (in_f // 2)],
                    )
                    halves.append(xi)
                return halves

            def make_tr_ops(halves, xt_t):
                # one closure per transpose; every 4th also drains psum -> xt
                ops = []
                state = {}
                half_k = KT // 2
                for b4 in range(KT // 4):
                    for j in range(4):
                        def op(b4=b4, j=j):
                            if j == 0:
                                state["ptr"] = ptr_pool.tile(
                                    [128, 512], f32, tag="ptr", name="ptr"
                                )
                            t = 4 * b4 + j
                            xi = halves[t // half_k]
                            tloc = t % half_k
                            nc.tensor.matmul(
                                state["ptr"][:, j * 128 : (j + 1) * 128],
                                xi[:, tloc * 128 : (tloc + 1) * 128],
                                ident[:],
                                is_transpose=True,
                            )
                            if j == 3:
                                nc.scalar.copy(
                                    xt_t[:, b4 * 512 : (b4 + 1) * 512],
                                    state["ptr"][:],
                                )
                        ops.append(op)
                return ops

            # prologue: m-tile 0 transposes up front
            halves0 = load_x(0)
            xt_cur = xt_pool.tile([128, in_f], f16, tag="xt", name="xt0")
            for op in make_tr_ops(halves0, xt_cur):
                op()

            for mi in range(MT):
                tr_ops = []
                if mi + 1 < MT:
                    halves_n = load_x(mi + 1)
                    xt_next = xt_pool.tile(
                        [128, in_f], f16, tag="xt", name=f"xt{mi + 1}"
                    )
                    tr_ops = make_tr_ops(halves_n, xt_next)
                else:
                    xt_next = None
                outt = out_pool.tile([128, out_shard], f32, tag="outt")
                n_mm = 0
                for (o, w) in chunks:
                    po = pout_pool.tile([128, w], f32, tag="po")
                    for t in range(KT):
                        nc.tensor.matmul(
                            po[:],
                            xt_cur[:, t * 128 : (t + 1) * 128],
                            w_tiles[t][:, o : o + w],
                            start=(t == 0),
                            stop=(t == KT - 1),
                        )
                        n_mm += 1
                        if n_mm % 3 == 0 and tr_ops:
                            tr_ops.pop(0)()
                    nc.scalar.copy(outt[:, o : o + w], po[:])
                while tr_ops:
                    tr_ops.pop(0)()
                nc.sync.dma_start(out_d[mi * 128 : (mi + 1) * 128, :], outt[:])
                xt_cur = xt_next

    nc.compile()
    return nc


_CACHE = {}


def _get_nc():
    if "nc" not in _CACHE:
        _CACHE["nc"] = build_nc()
    return _CACHE["nc"]


def shard_inputs(x, qweight, qzeros, scales):
    x = np.ascontiguousarray(np.asarray(x, dtype=np.float32).reshape(M_ROWS, IN_F))
    qweight = np.asarray(qweight)
    qzeros = np.asarray(qzeros)
    scales = np.asarray(scales)
    pz = OUT_SHARD // 8
    in_maps = []
    for c in range(N_CORES):
        lo, hi = c * OUT_SHARD, (c + 1) * OUT_SHARD
        in_maps.append(
            {
                "x": x,
                "qweight": np.repeat(qweight[:, lo:hi], 8, axis=0),
                "qzeros": np.ascontiguousarray(qzeros[:, c * pz : (c + 1) * pz]),
                "scales": np.repeat(scales[:, lo:hi], 128, axis=0),
            }
        )
    return in_maps


def gather_outputs(results):
    out = np.empty((M_ROWS, OUT_F), np.float32)
    for c in range(N_CORES):
        out[:, c * OUT_SHARD : (c + 1) * OUT_SHARD] = results[c]["out"]
    return out.reshape(B, S, OUT_F)


def kernel(x, qweight, qzeros, scales):
    in_maps = shard_inputs(x, qweight, qzeros, scales)
    res = run_bass_kernel_spmd(_get_nc(), in_maps, core_ids=list(range(N_CORES)))
    return gather_outputs(res.results)


# revision 22
# speedup vs baseline: 1.7669x; 1.7669x over previous
"""AutoRound GPTQ int4 linear on 8 TRN2 NeuronCores.

y = x @ dequant(qweight, qzeros, scales), column-parallel over out_features:
each core owns a [4096, 1376] weight shard, dequantizes it on-chip (int4
unpack + zero/scale in fp16), and runs an fp16 matmul with fp32 PSUM
accumulation. x is replicated; outputs are concatenated.

Host-side marshaling (layout only, no arithmetic): x is passed transposed
([in_f, m]) so the contraction dim lands on SBUF partitions directly;
qweight packed rows are repeated 8x so row k holds the int32 containing
weight row k; scales rows are repeated 128x so row k holds its group scale.

Device main loop is k-outer over 256-row m-blocks: per k, one [128, 256]
xT slice is cast to fp16 and used as the stationary operand of 6 matmuls
(2 m-tiles x 3 out-chunks) accumulating into 6 PSUM banks.
"""

import sys

sys.path.insert(0, "/opt/trn_rl_repo")

import numpy as np

import concourse.bacc as bacc
import concourse.mybir as mybir
import concourse.tile as tile
from concourse.bass_utils import run_bass_kernel_spmd

IN_F = 4096
OUT_F = 11008
G = 32  # quant groups (group size 128 == one k-tile)
N_CORES = 8
OUT_SHARD = OUT_F // N_CORES  # 1376
B, S = 4, 2048
M_ROWS = B * S  # 8192
M_BLK = 256

f32 = mybir.dt.float32
f16 = mybir.dt.float16
i32 = mybir.dt.int32
Alu = mybir.AluOpType


def build_nc(m_rows=M_ROWS, out_shard=OUT_SHARD, in_f=IN_F):
    KT = in_f // 128  # k-tiles; each k-tile is exactly one quant group
    NB = m_rows // M_BLK
    assert KT == G and m_rows % M_BLK == 0 and out_shard % 8 == 0

    chunks = []
    o = 0
    while o < out_shard:
        w = min(512, out_shard - o)
        chunks.append((o, w))
        o += w
    n_mt = M_BLK // 128  # m-tiles per block (2)

    nc = bacc.Bacc("TRN2", target_bir_lowering=False)
    xt_d = nc.dram_tensor("xt", (in_f, m_rows), f32, kind="ExternalInput")
    qw_d = nc.dram_tensor("qweight", (in_f, out_shard), i32, kind="ExternalInput")
    qz_d = nc.dram_tensor("qzeros", (G, out_shard // 8), i32, kind="ExternalInput")
    s_d = nc.dram_tensor("scales", (in_f, out_shard), f16, kind="ExternalInput")
    out_d = nc.dram_tensor("out", (m_rows, out_shard), f32, kind="ExternalOutput")

    with tile.TileContext(nc) as tc:
        with (
            tc.tile_pool(name="const", bufs=1) as cpool,
            tc.tile_pool(name="wpool", bufs=KT) as wpool,
            tc.tile_pool(name="qrep_p", bufs=3) as qrep_pool,
            tc.tile_pool(name="sb_p", bufs=3) as sb_pool,
            tc.tile_pool(name="row_p", bufs=3) as row_pool,
            tc.tile_pool(name="bcast_p", bufs=3) as bcast_pool,
            tc.tile_pool(name="xk_p", bufs=6) as xk_pool,
            tc.tile_pool(name="xkh_p", bufs=6) as xkh_pool,
            tc.tile_pool(name="out_p", bufs=4) as out_pool,
            tc.tile_pool(name="pout", bufs=8, space="PSUM") as pout_pool,
        ):
            # --- constants ---
            iota_t = cpool.tile([128, 1], i32, tag="iota")
            nc.gpsimd.iota(iota_t[:], pattern=[[0, 1]], base=0, channel_multiplier=4)
            # per-partition nibble shift: 4*(p % 8), int32 tensor operand
            shift_ap = cpool.tile([128, 1], i32, tag="shift")
            nc.vector.tensor_scalar(shift_ap[:], iota_t[:], 28, None, Alu.bitwise_and)

            qz_sb = cpool.tile([G, out_shard // 8], i32, tag="qz_sb")
            nc.sync.dma_start(qz_sb[:], qz_d[:])
            # unpack zeros along the free dim (int-only: bitvec ops cannot cast)
            z_sbi = cpool.tile([G, out_shard], i32, tag="z_sbi")
            z_r = z_sbi[:].rearrange("g (r i) -> g r i", i=8)
            for i in range(8):
                nc.vector.tensor_scalar(
                    z_r[:, :, i], qz_sb[:], 4 * i, 15,
                    Alu.logical_shift_right, Alu.bitwise_and,
                )
            z_sbh = cpool.tile([G, out_shard], f16, tag="z_sbh")
            nc.vector.tensor_copy(z_sbh[:], z_sbi[:])


            # --- dequantize weight shard into SBUF (fp16, [k, n] layout) ---
            w_tiles = []
            for t in range(KT):
                qrep = qrep_pool.tile([128, out_shard], i32, tag="qrep")
                nc.scalar.dma_start(qrep[:], qw_d[128 * t : 128 * (t + 1), :])
                sb = sb_pool.tile([128, out_shard], f16, tag="sb")
                nc.scalar.dma_start(sb[:], s_d[128 * t : 128 * (t + 1), :])
                zrow = row_pool.tile([1, out_shard], f16, tag="zrow")
                nc.sync.dma_start(zrow[:], z_sbh[t : t + 1, :])
                zb = bcast_pool.tile([128, out_shard], f16, tag="zb")
                nc.gpsimd.partition_broadcast(zb[:], zrow[:])
                # in-place int chain: q >>= shift; q &= 15
                nc.vector.tensor_tensor(
                    qrep[:], qrep[:],
                    shift_ap[:].broadcast_to((128, out_shard)),
                    Alu.logical_shift_right,
                )
                nc.vector.tensor_scalar(qrep[:], qrep[:], 15, None, Alu.bitwise_and)
                w_t = wpool.tile([128, out_shard], f16, tag="w")
                nc.scalar.copy(w_t[:], qrep[:])  # int32 -> fp16 (values 0..15)
                nc.vector.tensor_tensor(w_t[:], w_t[:], zb[:], Alu.subtract)
                nc.vector.tensor_tensor(w_t[:], w_t[:], sb[:], Alu.mult)
                w_tiles.append(w_t)

            # --- main loop: k-outer over 256-row m-blocks ---
            for mb in range(NB):
                m0 = mb * M_BLK
                pos = [
                    pout_pool.tile([128, w], f32, tag="po", name=f"po_{mb}_{j}_{ci}")
                    for j in range(n_mt)
                    for ci, (o, w) in enumerate(chunks)
                ]
                for t in range(KT):
                    xk = xk_pool.tile([128, M_BLK], f32, tag="xk")
                    nc.sync.dma_start(
                        xk[:], xt_d[t * 128 : (t + 1) * 128, m0 : m0 + M_BLK]
                    )
                    xkh = xkh_pool.tile([128, M_BLK], f16, tag="xkh")
                    nc.vector.tensor_copy(xkh[:], xk[:])
                    for j in range(n_mt):
                        for ci, (o, w) in enumerate(chunks):
                            nc.tensor.matmul(
                                pos[j * len(chunks) + ci][:],
                                xkh[:, j * 128 : (j + 1) * 128],
                                w_tiles[t][:, o : o + w],
                                start=(t == 0),
                                stop=(t == KT - 1),
                            )
                for j in range(n_mt):
                    outt = out_pool.tile([128, out_shard], f32, tag="outt")
                    for ci, (o, w) in enumerate(chunks):
                        nc.scalar.copy(
                            outt[:, o : o + w], pos[j * len(chunks) + ci][:]
                        )
                    nc.sync.dma_start(
                        out_d[m0 + j * 128 : m0 + (j + 1) * 128, :], outt[:]
                    )

    nc.compile()
    return nc


_CACHE = {}


def _get_nc():
    if "nc" not in _CACHE:
        _CACHE["nc"] = build_nc()
    return _CACHE["nc"]


def shard_inputs(x, qweight, qzeros, scales):
    x = np.asarray(x, dtype=np.float32).reshape(M_ROWS, IN_F)
    xt = np.ascontiguousarray(x.T)
    qweight = np.asarray(qweight)
    qzeros = np.asarray(qzeros)
    scales = np.asarray(scales)
    pz = OUT_SHARD // 8
    in_maps = []
    for c in range(N_CORES):
        lo, hi = c * OUT_SHARD, (c + 1) * OUT_SHARD
        in_maps.append(
            {
                "xt": xt,
                "qweight": np.repeat(qweight[:, lo:hi], 8, axis=0),
                "qzeros": np.ascontiguousarray(qzeros[:, c * pz : (c + 1) * pz]),
                "scales": np.repeat(scales[:, lo:hi], 128, axis=0),
            }
        )
    return in_maps


def gather_outputs(results):
    out = np.empty((M_ROWS, OUT_F), np.float32)
    for c in range(N_CORES):
        out[:, c * OUT_SHARD : (c + 1) * OUT_SHARD] = results[c]["out"]
    return out.reshape(B, S, OUT_F)


def kernel(x, qweight, qzeros, scales):
    in_maps = shard_inputs(x, qweight, qzeros, scales)
    res = run_bass_kernel_spmd(_get_nc(), in_maps, core_ids=list(range(N_CORES)))
    return gather_outputs(res.results)
